# revision 11
# baseline (speedup 1.0000x reference)
"""Bayesian linear layer (mean-field reparameterization) on 8 TRN2 NeuronCores.

out[b,o] = sum_i (eps_w[b,o,i]*exp(w_psi[o,i]) + w_mu[o,i]) * x[b,i]
         + eps_b[b,o]*exp(b_psi[o]) + b_mu[o]

Strategy (data-parallel over batch, 32 batches/core):
 - Host: transpose eps_w shard to [b, i, o], x shard to [i, b], psi/mu to
   [i, o]. All layout-only (no arithmetic on host).
 - Device per (b, i-chunk): DMA epsT tile [128i, 2x1024o]; DVE multiplies
   by resident sT = exp(psiT) into a float32r product tile; PE contracts
   over i with lhsT = x[:,b] column (matvec), accumulating the 8 i-chunks
   into a PSUM row [1, 1024].
 - mu term: single M=32 fp32 matmul xT @ muT accumulated in PSUM.
 - bias row: eps_b * exp(b_psi) + b_mu via partition_broadcast + DVE.
 - Assembly: ACT copies PSUM rows into a 32-aligned staging tile, 4 gather
   DMAs compact them to [32, 1024], DVE adds mu-term + bias, DMA out.
"""

import numpy as np

BS, OUT, IN = 256, 1024, 1024
NCORES = 8
BPC = BS // NCORES          # 32 batches per core
ICH = IN // 128             # 8 i-chunks
CPT = 2                     # i-chunks per DMA tile
OH = OUT // 512             # 2 output halves of 512

_cache = {}


def _build(reps, pe_mode, loop=False):
    import concourse.bass as bass
    import concourse.mybir as mybir
    import concourse.bacc as bacc
    from concourse import tile

    f32 = mybir.dt.float32
    f32r = mybir.dt.float32r
    bf16 = mybir.dt.bfloat16
    mult = mybir.AluOpType.mult
    add = mybir.AluOpType.add

    nc = bacc.Bacc(None, target_bir_lowering=False)

    d_eps = nc.dram_tensor("epsT", [BPC, IN, OUT], f32, kind="ExternalInput")
    d_xT = nc.dram_tensor("xT", [IN, BPC], f32, kind="ExternalInput")
    d_psiT = nc.dram_tensor("psiT", [IN, OUT], f32, kind="ExternalInput")
    d_muT = nc.dram_tensor("muT", [IN, OUT], f32, kind="ExternalInput")
    d_eb = nc.dram_tensor("eps_b", [BPC, OUT], f32, kind="ExternalInput")
    d_bpsi = nc.dram_tensor("bpsi", [1, OUT], f32, kind="ExternalInput")
    d_bmu = nc.dram_tensor("bmu", [1, OUT], f32, kind="ExternalInput")
    if loop:
        d_it = nc.dram_tensor("iters", [1, 1], mybir.dt.int32,
                              kind="ExternalInput")
    d_out = nc.dram_tensor("out", [BPC, OUT], f32, kind="ExternalOutput")

    with tile.TileContext(nc) as tc:
        with tc.tile_pool(name="const", bufs=1) as cpool, \
             tc.tile_pool(name="eps", bufs=5) as epool, \
             tc.tile_pool(name="p2b", bufs=5) as p2pool, \
             tc.tile_pool(name="ps", bufs=2, space="PSUM") as pspool, \
             tc.tile_pool(name="pst2", bufs=1, space="PSUM") as t2pool:

            p2dt = {"f32r": f32r, "f32": f32, "bf16": bf16}[pe_mode]
            sT = cpool.tile([128, ICH, OUT], f32, name="sT")
            xTf = cpool.tile([128, ICH, BPC], f32, name="xTf")
            xTm = cpool.tile([128, ICH, BPC], p2dt, name="xTm")
            ebt = cpool.tile([BPC, OUT], f32, name="ebt")
            sbrow = cpool.tile([1, OUT], f32, name="sbrow")
            sb_bc = cpool.tile([BPC, OUT], f32, name="sb_bc")
            mu_bc = cpool.tile([BPC, OUT], f32, name="mu_bc")
            bias_rows = cpool.tile([BPC, OUT], f32, name="bias_rows")
            staging = cpool.tile([128, (BPC // 4) * OUT], f32, name="staging")
            out_sb = cpool.tile([BPC, OUT], f32, name="out_sb")

            def emit(rep):
                # ---- prologue: params, exp(psi), bias rows, mu-term ----
                for ic in range(ICH):
                    pt = epool.tile([128, CPT, OUT], f32,
                                    name=f"psi_{rep}_{ic}", tag="eps")
                    nc.sync.dma_start(out=pt[:, 0, :],
                                      in_=d_psiT[ic * 128:(ic + 1) * 128, :])
                    nc.scalar.activation(sT[:, ic, :], pt[:, 0, :],
                                         mybir.ActivationFunctionType.Exp)
                    nc.sync.dma_start(out=xTf[:, ic, :],
                                      in_=d_xT[ic * 128:(ic + 1) * 128, :])
                nc.vector.tensor_copy(xTm[:], xTf[:])

                nc.sync.dma_start(out=ebt[:], in_=d_eb[:])
                nc.sync.dma_start(out=sbrow[:], in_=d_bpsi[:])
                nc.scalar.activation(sbrow[:], sbrow[:],
                                     mybir.ActivationFunctionType.Exp)
                nc.gpsimd.partition_broadcast(sb_bc[:], sbrow[:])
                murow = epool.tile([1, OUT], f32, name=f"murow_{rep}",
                                   tag="mur")
                nc.sync.dma_start(out=murow[:], in_=d_bmu[:])
                nc.gpsimd.partition_broadcast(mu_bc[:], murow[:])
                nc.vector.tensor_tensor(bias_rows[:], ebt[:], sb_bc[:], mult)
                nc.vector.tensor_tensor(bias_rows[:], bias_rows[:], mu_bc[:],
                                        add)

                # mu-term: t2[b, o] = sum_i x[b, i] * mu[o, i], M=32 fp32
                t2ps = t2pool.tile([BPC, OUT], f32, name=f"t2_{rep}", tag="t2")
                for t in range(ICH // CPT):
                    mt = epool.tile([128, CPT, OUT], f32,
                                    name=f"mu_{rep}_{t}", tag="eps")
                    nc.sync.dma_start(
                        out=mt[:],
                        in_=d_muT[t * CPT * 128:(t + 1) * CPT * 128, :]
                        .rearrange("(s p) o -> p s o", p=128))
                    for s in range(CPT):
                        ic = t * CPT + s
                        for h in range(OH):
                            nc.tensor.matmul(
                                t2ps[:, h * 512:(h + 1) * 512],
                                xTf[:, ic, :],
                                mt[:, s, h * 512:(h + 1) * 512],
                                start=(ic == 0), stop=(ic == ICH - 1))

                # ---- main loop: eps-term matvecs ----
                for b in range(BPC):
                    ps = pspool.tile([1, OUT], f32, name=f"ps_{rep}_{b}",
                                     tag="ps")
                    for t in range(ICH // CPT):
                        e = epool.tile([128, CPT, OUT], f32,
                                       name=f"e_{rep}_{b}_{t}", tag="eps")
                        nc.sync.dma_start(
                            out=e[:],
                            in_=d_eps[b, t * CPT * 128:(t + 1) * CPT * 128, :]
                            .rearrange("(s p) o -> p s o", p=128))
                        p2 = p2pool.tile([128, CPT, OUT], p2dt,
                                         name=f"p2_{rep}_{b}_{t}", tag="p2")
                        nc.vector.tensor_tensor(
                            p2[:], e[:], sT[:, t * CPT:(t + 1) * CPT, :], mult)
                        for s in range(CPT):
                            ic = t * CPT + s
                            for h in range(OH):
                                nc.tensor.matmul(
                                    ps[:, h * 512:(h + 1) * 512],
                                    xTm[:, ic, b:b + 1],
                                    p2[:, s, h * 512:(h + 1) * 512],
                                    start=(ic == 0), stop=(ic == ICH - 1))
                    # stage the finished row at a 32-aligned partition
                    j, g = b % 4, b // 4
                    nc.scalar.copy(
                        staging[32 * j:32 * j + 1, g * OUT:(g + 1) * OUT],
                        ps[:])

                # ---- epilogue: gather rows, add mu-term and bias ----
                for j in range(4):
                    nc.sync.dma_start(
                        out=out_sb[j:BPC:4, :],
                        in_=staging[32 * j:32 * j + 1, :])
                nc.vector.tensor_tensor(out_sb[:], out_sb[:], t2ps[:], add)
                nc.vector.tensor_tensor(out_sb[:], out_sb[:], bias_rows[:],
                                        add)
                nc.sync.dma_start(out=d_out[:], in_=out_sb[:])

            if loop:
                it_sb = cpool.tile([1, 1], mybir.dt.int32, name="it_sb")
                nc.sync.dma_start(out=it_sb[:], in_=d_it[:])
                regs = []
                for et in mybir.ALL_ENGINES:
                    eng = nc.engines[et]
                    r = eng.alloc_register(f"iters_{et.name}")
                    eng.reg_load(r, it_sb[0:1, 0:1])
                    regs.append(r)
                iters_val = bass.make_scalar_value(
                    bass.RegisterHandles(regs), min_val=1, max_val=1 << 20)
                with tc.For_i(0, iters_val, 1,
                              hint_engines=(mybir.EngineType.PE,
                                            mybir.EngineType.DVE,
                                            mybir.EngineType.SP)):
                    emit(0)
            else:
                for rep in range(reps):
                    emit(rep)

    nc.compile()
    return nc


def _get_nc(reps, pe_mode, loop=False):
    key = (reps, pe_mode, loop)
    if key not in _cache:
        _cache[key] = _build(reps, pe_mode, loop)
    return _cache[key]


def _prepare_inmaps(x, weight_mu, weight_psi, bias_mu, bias_psi, eps_w, eps_b):
    x = np.asarray(x, dtype=np.float32)
    weight_mu = np.asarray(weight_mu, dtype=np.float32)
    weight_psi = np.asarray(weight_psi, dtype=np.float32)
    bias_mu = np.asarray(bias_mu, dtype=np.float32)
    bias_psi = np.asarray(bias_psi, dtype=np.float32)
    eps_w = np.asarray(eps_w, dtype=np.float32)
    eps_b = np.asarray(eps_b, dtype=np.float32)

    psiT = np.ascontiguousarray(weight_psi.T)
    muT = np.ascontiguousarray(weight_mu.T)
    bpsi = bias_psi.reshape(1, OUT)
    bmu = bias_mu.reshape(1, OUT)

    in_maps = []
    for c in range(NCORES):
        sl = slice(c * BPC, (c + 1) * BPC)
        in_maps.append({
            "epsT": np.ascontiguousarray(eps_w[sl].transpose(0, 2, 1)),
            "xT": np.ascontiguousarray(x[sl].T),
            "psiT": psiT,
            "muT": muT,
            "eps_b": np.ascontiguousarray(eps_b[sl]),
            "bpsi": bpsi,
            "bmu": bmu,
        })
    return in_maps


def _run(in_maps, reps=1, pe_mode="f32r", loop_iters=None):
    from concourse.bass_utils import run_bass_kernel_spmd
    nc = _get_nc(reps, pe_mode, loop=loop_iters is not None)
    if loop_iters is not None:
        it = np.array([[loop_iters]], dtype=np.int32)
        in_maps = [{**m, "iters": it} for m in in_maps]
    res = run_bass_kernel_spmd(nc, in_maps, core_ids=list(range(NCORES)))
    return np.concatenate([res.results[c]["out"] for c in range(NCORES)],
                          axis=0)


def kernel(x, weight_mu, weight_psi, bias_mu, bias_psi, eps_w, eps_b,
           _pe_mode="f32r"):
    in_maps = _prepare_inmaps(x, weight_mu, weight_psi, bias_mu, bias_psi,
                              eps_w, eps_b)
    return _run(in_maps, pe_mode=_pe_mode)


# revision 13
# speedup vs baseline: 1.4879x; 1.4879x over previous
"""Bayesian linear layer (mean-field reparameterization) on 8 TRN2 NeuronCores.

out[b,o] = sum_i (eps_w[b,o,i]*exp(w_psi[o,i]) + w_mu[o,i]) * x[b,i]
         + eps_b[b,o]*exp(b_psi[o]) + b_mu[o]

Strategy (data-parallel over batch, 32 batches/core):
 - Host: transpose eps_w shard to [b, i, o], x shard to [i, b], psi/mu to
   [i, o]. All layout-only (no arithmetic on host).
 - Device per (b, i-chunk): DMA epsT tile [128i, 2x1024o]; DVE multiplies
   by resident sT = exp(psiT) into a float32r product tile; PE contracts
   over i with lhsT = x[:,b] column (matvec), accumulating the 8 i-chunks
   into a PSUM row [1, 1024].
 - mu term: single M=32 fp32 matmul xT @ muT accumulated in PSUM.
 - bias row: eps_b * exp(b_psi) + b_mu via partition_broadcast + DVE.
 - Assembly: ACT copies PSUM rows into a 32-aligned staging tile, 4 gather
   DMAs compact them to [32, 1024], DVE adds mu-term + bias, DMA out.
"""

import numpy as np

import os

BS, OUT, IN = 256, 1024, 1024
NCORES = 8
BPC = BS // NCORES          # 32 batches per core
ICH = IN // 128             # 8 i-chunks
CPT = int(os.environ.get("BK_CPT", "2"))   # i-chunks per DMA tile
EBUFS = int(os.environ.get("BK_EBUFS", "5"))
PBUFS = int(os.environ.get("BK_PBUFS", "5"))
OH = OUT // 512             # 2 output halves of 512

_cache = {}


def _build(reps, pe_mode, loop=False):
    import concourse.bass as bass
    import concourse.mybir as mybir
    import concourse.bacc as bacc
    from concourse import tile

    f32 = mybir.dt.float32
    f32r = mybir.dt.float32r
    bf16 = mybir.dt.bfloat16
    mult = mybir.AluOpType.mult
    add = mybir.AluOpType.add

    nc = bacc.Bacc(None, target_bir_lowering=False)

    d_eps = nc.dram_tensor("epsT", [BPC, IN, OUT], f32, kind="ExternalInput")
    d_xT = nc.dram_tensor("xT", [IN, BPC], f32, kind="ExternalInput")
    d_psiT = nc.dram_tensor("psiT", [IN, OUT], f32, kind="ExternalInput")
    d_muT = nc.dram_tensor("muT", [IN, OUT], f32, kind="ExternalInput")
    d_eb = nc.dram_tensor("eps_b", [BPC, OUT], f32, kind="ExternalInput")
    d_bpsi = nc.dram_tensor("bpsi", [1, OUT], f32, kind="ExternalInput")
    d_bmu = nc.dram_tensor("bmu", [1, OUT], f32, kind="ExternalInput")
    if loop:
        d_it = nc.dram_tensor("iters", [1, 1], mybir.dt.int32,
                              kind="ExternalInput")
    d_out = nc.dram_tensor("out", [BPC, OUT], f32, kind="ExternalOutput")

    with tile.TileContext(nc) as tc:
        with tc.tile_pool(name="const", bufs=1) as cpool, \
             tc.tile_pool(name="eps", bufs=EBUFS) as epool, \
             tc.tile_pool(name="p2b", bufs=PBUFS) as p2pool, \
             tc.tile_pool(name="ps", bufs=2, space="PSUM") as pspool, \
             tc.tile_pool(name="pst2", bufs=1, space="PSUM") as t2pool:

            p2dt = {"f32r": f32r, "f32": f32, "bf16": bf16}[pe_mode]
            sT = cpool.tile([128, ICH, OUT], f32, name="sT")
            xTf = cpool.tile([128, ICH, BPC], f32, name="xTf")
            xTm = cpool.tile([128, ICH, BPC], p2dt, name="xTm")
            ebt = cpool.tile([BPC, OUT], f32, name="ebt")
            sbrow = cpool.tile([1, OUT], f32, name="sbrow")
            sb_bc = cpool.tile([BPC, OUT], f32, name="sb_bc")
            mu_bc = cpool.tile([BPC, OUT], f32, name="mu_bc")
            bias_rows = cpool.tile([BPC, OUT], f32, name="bias_rows")
            staging = cpool.tile([128, (BPC // 4) * OUT], f32, name="staging")
            out_sb = cpool.tile([BPC, OUT], f32, name="out_sb")

            def emit(rep):
                # ---- prologue: params, exp(psi), bias rows, mu-term ----
                for ic in range(ICH):
                    pt = epool.tile([128, CPT, OUT], f32,
                                    name=f"psi_{rep}_{ic}", tag="eps")
                    nc.sync.dma_start(out=pt[:, 0, :],
                                      in_=d_psiT[ic * 128:(ic + 1) * 128, :])
                    nc.scalar.activation(sT[:, ic, :], pt[:, 0, :],
                                         mybir.ActivationFunctionType.Exp)
                    nc.sync.dma_start(out=xTf[:, ic, :],
                                      in_=d_xT[ic * 128:(ic + 1) * 128, :])
                nc.vector.tensor_copy(xTm[:], xTf[:])

                nc.sync.dma_start(out=ebt[:], in_=d_eb[:])
                nc.sync.dma_start(out=sbrow[:], in_=d_bpsi[:])
                nc.scalar.activation(sbrow[:], sbrow[:],
                                     mybir.ActivationFunctionType.Exp)
                nc.gpsimd.partition_broadcast(sb_bc[:], sbrow[:])
                murow = epool.tile([1, OUT], f32, name=f"murow_{rep}",
                                   tag="mur")
                nc.sync.dma_start(out=murow[:], in_=d_bmu[:])
                nc.gpsimd.partition_broadcast(mu_bc[:], murow[:])
                nc.vector.tensor_tensor(bias_rows[:], ebt[:], sb_bc[:], mult)
                nc.vector.tensor_tensor(bias_rows[:], bias_rows[:], mu_bc[:],
                                        add)

                # mu-term: t2[b, o] = sum_i x[b, i] * mu[o, i], M=32 fp32
                t2ps = t2pool.tile([BPC, OUT], f32, name=f"t2_{rep}", tag="t2")
                for t in range(ICH // CPT):
                    mt = epool.tile([128, CPT, OUT], f32,
                                    name=f"mu_{rep}_{t}", tag="eps")
                    nc.sync.dma_start(
                        out=mt[:],
                        in_=d_muT[t * CPT * 128:(t + 1) * CPT * 128, :]
                        .rearrange("(s p) o -> p s o", p=128))
                    for s in range(CPT):
                        ic = t * CPT + s
                        for h in range(OH):
                            nc.tensor.matmul(
                                t2ps[:, h * 512:(h + 1) * 512],
                                xTf[:, ic, :],
                                mt[:, s, h * 512:(h + 1) * 512],
                                start=(ic == 0), stop=(ic == ICH - 1))

                # ---- main loop: eps-term matvecs ----
                for b in range(BPC):
                    ps = pspool.tile([1, OUT], f32, name=f"ps_{rep}_{b}",
                                     tag="ps")
                    for t in range(ICH // CPT):
                        e = epool.tile([128, CPT, OUT], f32,
                                       name=f"e_{rep}_{b}_{t}", tag="eps")
                        nc.sync.dma_start(
                            out=e[:],
                            in_=d_eps[b, t * CPT * 128:(t + 1) * CPT * 128, :]
                            .rearrange("(s p) o -> p s o", p=128))
                        p2 = p2pool.tile([128, CPT, OUT], p2dt,
                                         name=f"p2_{rep}_{b}_{t}", tag="p2")
                        nc.vector.tensor_tensor(
                            p2[:], e[:], sT[:, t * CPT:(t + 1) * CPT, :], mult)
                        for s in range(CPT):
                            ic = t * CPT + s
                            for h in range(OH):
                                nc.tensor.matmul(
                                    ps[:, h * 512:(h + 1) * 512],
                                    xTm[:, ic, b:b + 1],
                                    p2[:, s, h * 512:(h + 1) * 512],
                                    start=(ic == 0), stop=(ic == ICH - 1))
                    # stage the finished row at a 32-aligned partition
                    j, g = b % 4, b // 4
                    nc.scalar.copy(
                        staging[32 * j:32 * j + 1, g * OUT:(g + 1) * OUT],
                        ps[:])

                # ---- epilogue: gather rows, add mu-term and bias ----
                for j in range(4):
                    nc.sync.dma_start(
                        out=out_sb[j:BPC:4, :],
                        in_=staging[32 * j:32 * j + 1, :])
                nc.vector.tensor_tensor(out_sb[:], out_sb[:], t2ps[:], add)
                nc.vector.tensor_tensor(out_sb[:], out_sb[:], bias_rows[:],
                                        add)
                nc.sync.dma_start(out=d_out[:], in_=out_sb[:])

            if loop:
                it_sb = cpool.tile([1, 1], mybir.dt.int32, name="it_sb")
                nc.sync.dma_start(out=it_sb[:], in_=d_it[:])
                regs = []
                for et in mybir.ALL_ENGINES:
                    eng = nc.engines[et]
                    r = eng.alloc_register(f"iters_{et.name}")
                    eng.reg_load(r, it_sb[0:1, 0:1])
                    regs.append(r)
                iters_val = bass.make_scalar_value(
                    bass.RegisterHandles(regs), min_val=1, max_val=1 << 20)
                with tc.For_i(0, iters_val, 1,
                              hint_engines=(mybir.EngineType.PE,
                                            mybir.EngineType.DVE,
                                            mybir.EngineType.SP)):
                    emit(0)
            else:
                for rep in range(reps):
                    emit(rep)

    nc.compile()
    return nc


def _get_nc(reps, pe_mode, loop=False):
    key = (reps, pe_mode, loop)
    if key not in _cache:
        _cache[key] = _build(reps, pe_mode, loop)
    return _cache[key]


def _prepare_inmaps(x, weight_mu, weight_psi, bias_mu, bias_psi, eps_w, eps_b):
    x = np.asarray(x, dtype=np.float32)
    weight_mu = np.asarray(weight_mu, dtype=np.float32)
    weight_psi = np.asarray(weight_psi, dtype=np.float32)
    bias_mu = np.asarray(bias_mu, dtype=np.float32)
    bias_psi = np.asarray(bias_psi, dtype=np.float32)
    eps_w = np.asarray(eps_w, dtype=np.float32)
    eps_b = np.asarray(eps_b, dtype=np.float32)

    psiT = np.ascontiguousarray(weight_psi.T)
    muT = np.ascontiguousarray(weight_mu.T)
    bpsi = bias_psi.reshape(1, OUT)
    bmu = bias_mu.reshape(1, OUT)

    in_maps = []
    for c in range(NCORES):
        sl = slice(c * BPC, (c + 1) * BPC)
        in_maps.append({
            "epsT": np.ascontiguousarray(eps_w[sl].transpose(0, 2, 1)),
            "xT": np.ascontiguousarray(x[sl].T),
            "psiT": psiT,
            "muT": muT,
            "eps_b": np.ascontiguousarray(eps_b[sl]),
            "bpsi": bpsi,
            "bmu": bmu,
        })
    return in_maps


def _run(in_maps, reps=1, pe_mode="f32r", loop_iters=None):
    from concourse.bass_utils import run_bass_kernel_spmd
    nc = _get_nc(reps, pe_mode, loop=loop_iters is not None)
    if loop_iters is not None:
        it = np.array([[loop_iters]], dtype=np.int32)
        in_maps = [{**m, "iters": it} for m in in_maps]
    res = run_bass_kernel_spmd(nc, in_maps, core_ids=list(range(NCORES)))
    return np.concatenate([res.results[c]["out"] for c in range(NCORES)],
                          axis=0)


def kernel(x, weight_mu, weight_psi, bias_mu, bias_psi, eps_w, eps_b,
           _pe_mode="f32r"):
    in_maps = _prepare_inmaps(x, weight_mu, weight_psi, bias_mu, bias_psi,
                              eps_w, eps_b)
    return _run(in_maps, pe_mode=_pe_mode)
